# revision 11
# baseline (speedup 1.0000x reference)
"""Causal self-attention on 8 trn2 NeuronCores.

Sharding: head-parallel QKV+attention (core c owns heads {2c, 2c+1} of both
batches over the full sequence), then two per-batch AllToAlls re-shard to
token-parallel for the output projection (core c owns tokens
[256c, 256c+256) of each batch).

All matmuls run in bf16 (same PE rate as f32r on TRN2, half the DMA/SBUF
bytes) with fp32 PSUM accumulation. The attention AV matmul is swapped
(stationary = probs block, moving = V augmented with a ones column) so each
[128q x 129] output tile carries its own softmax row-sums in column 128 —
no separate row-sum matmuls — and normalization is a per-partition
tensor_scalar. The causal structure is exploited at 128-query granularity
on the diagonal; the strictly-lower region runs at full 512-query width.

The two collectives overlap compute: batch 0's AllToAll runs under batch
1's QKV+attention, batch 1's under batch 0's projection half (w_proj is
SBUF-resident). Projection input arrives via XBAR DMA-transpose directly in
[channel, token] layout.
"""

import sys

sys.path.insert(0, "/opt/trn_rl_repo")

import numpy as np
import ml_dtypes

import concourse.bass as bass
import concourse.mybir as mybir
import concourse.tile as tile
from concourse.bass_utils import run_bass_kernel_spmd

N_CORES = 8
B, T, C = 2, 2048, 2048
NH, HD = 16, 128
P = 128
KC = C // P            # 16 contraction subtiles
NB = 4                 # 512-wide t-chunks per batch
QC = 4                 # 512-wide q-chunks per batch
HL = 2                 # heads per core
BT = B * T             # 4096
TSL = 256              # tokens per core per batch (proj shard)

f32 = mybir.dt.float32
bf16 = mybir.dt.bfloat16
ACTF = mybir.ActivationFunctionType
ALU = mybir.AluOpType
bfnp = ml_dtypes.bfloat16

_CACHE = {}


def _split_multi_waits(nc, max_waits=1):
    """This container's walrus rejects >1 sync-wait per instruction; hoist
    extra waits onto same-engine NoOps placed just before the instruction."""
    n_split = 0
    for fn in nc.m.functions:
        for bb in fn.blocks:
            insts = list(bb.instructions)
            out = []
            changed = False
            for inst in insts:
                si = inst.sync_info
                waits = list(si.on_wait) if (si is not None and si.on_wait) else []
                if len(waits) > max_waits:
                    ups = list(si.on_update) if si.on_update else []
                    head, tail = waits[:-max_waits], waits[-max_waits:]
                    for i, w in enumerate(head):
                        nop = mybir.InstNoOp(name=f"{inst.name}-wsplit-{i}")
                        nop.engine = inst.engine
                        nop.sync_info = mybir.SyncInfo(on_wait=[w], on_update=[])
                        out.append(nop)
                    inst.sync_info = mybir.SyncInfo(on_wait=tail, on_update=ups)
                    changed = True
                    n_split += 1
                out.append(inst)
            if changed:
                bb.instructions = out
    return n_split


def _build_bass(repeat=1, with_att=True, with_collective=True, with_proj=True,
                debug_a2a=False):
    nc = bass.Bass("TRN2", target_bir_lowering=False, debug=False,
                   num_devices=N_CORES)

    xT = nc.declare_dram_parameter("xT", [C, BT], bf16, isOutput=False)
    w_qkv = nc.declare_dram_parameter("w_qkv", [C, 3 * HL * HD], bf16,
                                      isOutput=False)
    b_qk = nc.declare_dram_parameter("b_qk", [2 * HL * HD], f32,
                                     isOutput=False)
    b_v = nc.declare_dram_parameter("b_v", [1, HL * HD], bf16, isOutput=False)
    w_proj = nc.declare_dram_parameter("w_proj", [C, C], bf16, isOutput=False)
    b_proj = nc.declare_dram_parameter("b_proj", [C], f32, isOutput=False)
    tri = nc.declare_dram_parameter("tri", [P, P], bf16, isOutput=False)
    ones_c = nc.declare_dram_parameter("ones_c", [1, P], bf16, isOutput=False)
    outT = nc.declare_dram_parameter("outT", [C, B * TSL], f32, isOutput=True)
    if debug_a2a:
        dbg = nc.declare_dram_parameter("dbg", [B * N_CORES * TSL, HL * HD],
                                        bf16, isOutput=True)

    xT_t = xT.rearrange("(kc p) t -> p kc t", p=P)          # [128,16,4096]
    wq_t = w_qkv.rearrange("(kc p) n -> p kc n", p=P)       # [128,16,768]
    wp_t = w_proj.rearrange("(kc p) n -> p kc n", p=P)      # [128,16,2048]

    scale = float(HD) ** -0.5

    with tile.TileContext(nc) as tc:
        with (
            # attention PSUM (banks 0-3): scores 2 + paired-o accum 2
            tc.tile_pool(name="attps", bufs=1, space="PSUM") as attps,
            tc.tile_pool(name="sb", bufs=1) as sb,
            tc.tile_pool(name="dram", bufs=1, space="DRAM") as dram,
        ):
            # ---- warmup-critical prefetch order: first x chunk, then the
            # first weight column, then everything else ----
            xc0 = sb.tile([P, KC, 512], bf16, tag="xc", bufs=2)
            for dq4 in range(4):
                nc.sync.dma_start(xc0[:, dq4 * 4:(dq4 + 1) * 4, :],
                                  xT_t[:, dq4 * 4:(dq4 + 1) * 4, 0:512])
            wq_sb = sb.tile([P, KC, 3 * HL * HD], bf16, tag="wq")
            for dq6 in range(6):
                nc.sync.dma_start(wq_sb[:, :, dq6 * P:(dq6 + 1) * P],
                                  wq_t[:, :, dq6 * P:(dq6 + 1) * P])
            bqk_sb = sb.tile([P, 4], f32, tag="bqk")         # q/k bias per col
            nc.sync.dma_start(bqk_sb[:],
                              b_qk.rearrange("(m p) -> p m", p=P))
            bv_sb = sb.tile([1, HL * HD], bf16, tag="bv")    # v bias row
            nc.sync.dma_start(bv_sb[:], b_v[:, :])
            bp_sb = sb.tile([P, KC], f32, tag="bp")          # proj bias
            nc.sync.dma_start(bp_sb[:], b_proj.rearrange("(m p) -> p m", p=P))
            tri_sb = sb.tile([P, P], bf16, tag="tri")        # diag mask
            nc.sync.dma_start(tri_sb[:], tri[:, :])
            onesc_sb = sb.tile([1, P], bf16, tag="onesc")
            nc.sync.dma_start(onesc_sb[:], ones_c[:, :])
            wp_sb = sb.tile([P, KC, C], bf16, tag="wp")      # resident w_proj

            a2a_in = [dram.tile([N_CORES, TSL, HL * HD], bf16, tag=f"ain{b}",
                                name=f"a2a_in{b}")
                      for b in range(B)]
            a2a_out = [dram.tile([N_CORES, TSL, HL * HD], bf16,
                                 tag=f"aout{b}", name=f"a2a_out{b}")
                       for b in range(B)]

            def emit_att(b, hl, qc, qk_sb, v_sb):
                qT_h = qk_sb[:, hl]              # [128, 2048]
                kT_h = qk_sb[:, 2 + hl]
                o4 = [attps.tile([P, HD + 1], f32, tag="o", bufs=4,
                                 name=f"o{jj}") for jj in range(4)]
                # units: (kb, wstart, first-diag?)
                units = [(kb, 0, False) for kb in range(4 * qc)]
                units += [(4 * qc + i, i * P, True) for i in range(4)]

                def emit_scores(u):
                    kb, ws, diag = u
                    s = attps.tile([P, 512], f32, tag="s", bufs=2)
                    nc.tensor.matmul(
                        s[:, ws:512],
                        kT_h[:, kb * P:(kb + 1) * P],
                        qT_h[:, qc * 512 + ws:(qc + 1) * 512],
                        start=True, stop=True)
                    probs = sb.tile([P, 512], bf16, tag="probs", bufs=3)
                    nc.scalar.activation(
                        out=probs[:, ws:512], in_=s[:, ws:512],
                        func=ACTF.Exp, scale=scale)
                    if diag:
                        nc.vector.tensor_tensor(
                            out=probs[:, ws:ws + P], in0=probs[:, ws:ws + P],
                            in1=tri_sb[:], op=ALU.mult)
                    return probs

                def emit_av(u, probs):
                    kb, ws, diag = u
                    j0 = ws // P if diag else 0
                    for j in range(j0, 4):
                        nc.tensor.matmul(
                            o4[j],
                            probs[:, j * P:(j + 1) * P],
                            v_sb[:, kb, hl, :],
                            start=(kb == 0),
                            stop=(kb == 4 * qc + j))

                prev = None
                for u in units:
                    probs = emit_scores(u)
                    if prev is not None:
                        emit_av(prev[0], prev[1])
                    prev = (u, probs)
                emit_av(prev[0], prev[1])

                ost = sb.tile([P, 4, HD], bf16, tag="ost", bufs=2)
                for j in range(4):
                    recip = sb.tile([P, 1], f32, tag="recip", bufs=4)
                    nc.vector.reciprocal(recip[:], o4[j][:, HD:HD + 1])
                    nc.vector.tensor_scalar_mul(
                        out=ost[:, j, :], in0=o4[j][:, 0:HD],
                        scalar1=recip[:])
                # one batched store: j -> chunk 2qc+j//2, row (j%2)*128
                dst = a2a_in[b].rearrange(
                    "(c2 c1) (t1 p) d -> p c2 c1 t1 d", c1=2, t1=2)
                nc.gpsimd.dma_start(
                    dst[:, qc, :, :, hl * HD:(hl + 1) * HD],
                    ost.rearrange("p (c1 t1) d -> p c1 t1 d", c1=2))

            for _rep in range(repeat):
              for b in range(B):
                qk_sb = sb.tile([P, 4, T], bf16, tag="qk", bufs=2)
                v_sb = sb.tile([P, KC, HL, HD + 1], bf16, tag="v", bufs=2)
                nc.vector.memset(v_sb[:, :, :, HD:HD + 1], 1.0)

                # ---- QKV projection interleaved with attention (qc = nb) ----
                with (
                    tc.tile_pool(name=f"qkv_ps{b}", bufs=2, space="PSUM") as qkvps,
                ):
                    for nb in range(NB):
                        g = b * T + nb * 512
                        if _rep == 0 and b == 0 and nb == 0:
                            xc = xc0        # prefetched before the weights
                        else:
                            xc = sb.tile([P, KC, 512], bf16, tag="xc", bufs=2)
                            for dq4 in range(4):
                                nc.sync.dma_start(
                                    xc[:, dq4 * 4:(dq4 + 1) * 4, :],
                                    xT_t[:, dq4 * 4:(dq4 + 1) * 4, g:g + 512])
                        for m in range(4):               # qh0 qh1 kh0 kh1
                            ps = qkvps.tile([P, 512], f32, tag="ps")
                            for kc in range(KC):
                                nc.tensor.matmul(
                                    ps[:],
                                    wq_sb[:, kc, m * P:(m + 1) * P],
                                    xc[:, kc, :],
                                    start=(kc == 0), stop=(kc == KC - 1))
                            nc.vector.tensor_scalar_add(
                                out=qk_sb[:, m, nb * 512:(nb + 1) * 512],
                                in0=ps[:], scalar1=bqk_sb[:, m:m + 1])
                        for tv in range(4):
                            ps = qkvps.tile([P, HL * HD], f32, tag="ps")
                            for kc in range(KC):
                                nc.tensor.matmul(
                                    ps[:],
                                    xc[:, kc, tv * P:(tv + 1) * P],
                                    wq_sb[:, kc, 2 * HL * HD:3 * HL * HD],
                                    start=(kc == 0), stop=False)
                            nc.tensor.matmul(
                                ps[:], onesc_sb[:], bv_sb[:],
                                start=False, stop=True)
                            # [tok,256] -> v_sb [tok, head, 0:128]
                            nc.vector.tensor_copy(
                                v_sb[:, nb * 4 + tv, :, 0:HD], ps[:])
                        if _rep == 0 and b == 0 and nb == 1:
                            # w_proj load rides under early batch-0 compute
                            for dq4 in range(4):
                                nc.sync.dma_start(
                                    wp_sb[:, dq4 * 4:(dq4 + 1) * 4, :],
                                    wp_t[:, dq4 * 4:(dq4 + 1) * 4, :])
                        if with_att:
                            for hl in range(HL):
                                emit_att(b, hl, nb, qk_sb, v_sb)

                # ---- head exchange for this batch ----
                if debug_a2a:
                    nc.sync.dma_start(
                        dbg[b * N_CORES * TSL:(b + 1) * N_CORES * TSL, :],
                        a2a_in[b].rearrange("c t d -> (c t) d"))
                if with_collective:
                    nc.gpsimd.collective_compute(
                        "AllToAll", ALU.bypass,
                        replica_groups=[list(range(N_CORES))],
                        ins=[a2a_in[b].opt()], outs=[a2a_out[b].opt()])

              # ---- output projection on own token slices ----
              if not with_proj:
                  continue
              for b in range(B):
                  with tc.tile_pool(name=f"proj{b}", bufs=2,
                                    space="PSUM") as pps:
                      rhs_sb = sb.tile([P, KC, TSL], bf16, tag="rhs", bufs=2)
                      for p8 in range(N_CORES):
                          for h2 in range(HL):
                              nc.sync.dma_start(
                                  rhs_sb[:, HL * p8 + h2, :],
                                  a2a_out[b][p8, :, h2 * HD:(h2 + 1) * HD],
                                  transpose=True)
                      for m in range(KC):
                          ps = pps.tile([P, TSL], f32)
                          for kc in range(KC):
                              nc.tensor.matmul(ps[:], wp_sb[:, kc,
                                                            m * P:(m + 1) * P],
                                               rhs_sb[:, kc, :],
                                               start=(kc == 0),
                                               stop=(kc == KC - 1))
                          o = sb.tile([P, TSL], f32, tag="pout", bufs=2)
                          nc.scalar.activation(out=o[:], in_=ps[:],
                                               func=ACTF.Identity,
                                               bias=bp_sb[:, m:m + 1],
                                               scale=1.0)
                          nc.sync.dma_start(
                              outT[m * P:(m + 1) * P, b * TSL:(b + 1) * TSL],
                              o[:])

    _split_multi_waits(nc)
    return nc


def _host_inputs(x, w_attn, b_attn, w_proj, b_proj):
    x = np.asarray(x, dtype=np.float32)
    w_attn = np.asarray(w_attn, dtype=np.float32)
    b_attn = np.asarray(b_attn, dtype=np.float32)
    w_proj = np.asarray(w_proj, dtype=np.float32)
    b_proj = np.ascontiguousarray(np.asarray(b_proj, dtype=np.float32))

    xT = np.ascontiguousarray(x.reshape(BT, C).T.astype(bfnp))
    w_proj_bf = np.ascontiguousarray(w_proj.astype(bfnp))

    # tri[r, c] = 1 where query col >= key row (within a diagonal block)
    cols = np.arange(P)
    tri = (cols[None, :] >= cols[:, None]).astype(bfnp)
    ones_c = np.ones((1, P), dtype=bfnp)

    in_maps = []
    for c in range(N_CORES):
        col = HL * c * HD
        w_qkv = np.concatenate(
            [w_attn[:, col:col + HL * HD],
             w_attn[:, C + col:C + col + HL * HD],
             w_attn[:, 2 * C + col:2 * C + col + HL * HD]], axis=1)
        b_qk = np.concatenate(
            [b_attn[col:col + HL * HD],
             b_attn[C + col:C + col + HL * HD]])
        b_v = b_attn[2 * C + col:2 * C + col + HL * HD]
        in_maps.append({
            "xT": xT,
            "w_qkv": np.ascontiguousarray(w_qkv.astype(bfnp)),
            "b_qk": np.ascontiguousarray(b_qk),
            "b_v": np.ascontiguousarray(b_v[None, :].astype(bfnp)),
            "w_proj": w_proj_bf,
            "b_proj": b_proj,
            "tri": tri,
            "ones_c": ones_c,
        })
    return in_maps


def kernel(x, w_attn, b_attn, w_proj, b_proj, _results_out=None):
    if "nc" not in _CACHE:
        _CACHE["nc"] = _build_bass()
    nc = _CACHE["nc"]
    in_maps = _host_inputs(x, w_attn, b_attn, w_proj, b_proj)
    res = run_bass_kernel_spmd(nc, in_maps, list(range(N_CORES)))
    if _results_out is not None:
        _results_out.append(res)
    out = np.empty((B, T, C), dtype=np.float32)
    for c in range(N_CORES):
        oc = res.results[c]["outT"]                      # [C, 2*256]
        for b in range(B):
            out[b, c * TSL:(c + 1) * TSL, :] = oc[:, b * TSL:(b + 1) * TSL].T
    return out


# revision 12
# speedup vs baseline: 1.8018x; 1.8018x over previous
"""Causal self-attention on 8 trn2 NeuronCores.

Sharding: head-parallel QKV+attention (core c owns heads {2c, 2c+1} of both
batches over the full sequence), then two per-batch AllToAlls re-shard to
token-parallel for the output projection (core c owns tokens
[256c, 256c+256) of each batch).

All matmuls run in bf16 (same PE rate as f32r on TRN2, half the DMA/SBUF
bytes) with fp32 PSUM accumulation. The attention AV matmul is swapped
(stationary = probs block, moving = V augmented with a ones column) so each
[128q x 129] output tile carries its own softmax row-sums in column 128 —
no separate row-sum matmuls — and normalization is a per-partition
tensor_scalar. The causal structure is exploited at 128-query granularity
on the diagonal; the strictly-lower region runs at full 512-query width.

The two collectives overlap compute: batch 0's AllToAll runs under batch
1's QKV+attention, batch 1's under batch 0's projection half (w_proj is
SBUF-resident). Projection input arrives via XBAR DMA-transpose directly in
[channel, token] layout.
"""

import sys

sys.path.insert(0, "/opt/trn_rl_repo")

import numpy as np
import ml_dtypes

import concourse.bass as bass
import concourse.mybir as mybir
import concourse.tile as tile
from concourse.bass_utils import run_bass_kernel_spmd

N_CORES = 8
B, T, C = 2, 2048, 2048
NH, HD = 16, 128
P = 128
KC = C // P            # 16 contraction subtiles
NB = 4                 # 512-wide t-chunks per batch
QC = 4                 # 512-wide q-chunks per batch
HL = 2                 # heads per core
BT = B * T             # 4096
TSL = 256              # tokens per core per batch (proj shard)

f32 = mybir.dt.float32
bf16 = mybir.dt.bfloat16
ACTF = mybir.ActivationFunctionType
ALU = mybir.AluOpType
bfnp = ml_dtypes.bfloat16

_CACHE = {}


def _split_multi_waits(nc, max_waits=1):
    """This container's walrus rejects >1 sync-wait per instruction; hoist
    extra waits onto same-engine NoOps placed just before the instruction."""
    n_split = 0
    for fn in nc.m.functions:
        for bb in fn.blocks:
            insts = list(bb.instructions)
            out = []
            changed = False
            for inst in insts:
                si = inst.sync_info
                waits = list(si.on_wait) if (si is not None and si.on_wait) else []
                if len(waits) > max_waits:
                    ups = list(si.on_update) if si.on_update else []
                    head, tail = waits[:-max_waits], waits[-max_waits:]
                    for i, w in enumerate(head):
                        nop = mybir.InstNoOp(name=f"{inst.name}-wsplit-{i}")
                        nop.engine = inst.engine
                        nop.sync_info = mybir.SyncInfo(on_wait=[w], on_update=[])
                        out.append(nop)
                    inst.sync_info = mybir.SyncInfo(on_wait=tail, on_update=ups)
                    changed = True
                    n_split += 1
                out.append(inst)
            if changed:
                bb.instructions = out
    return n_split


def _build_bass(repeat=1, with_att=True, with_collective=True, with_proj=True,
                debug_a2a=False):
    nc = bass.Bass("TRN2", target_bir_lowering=False, debug=False,
                   num_devices=N_CORES)

    xT = nc.declare_dram_parameter("xT", [C, BT], bf16, isOutput=False)
    w_qkv = nc.declare_dram_parameter("w_qkv", [C, 3 * HL * HD], bf16,
                                      isOutput=False)
    b_qk = nc.declare_dram_parameter("b_qk", [2 * HL * HD], f32,
                                     isOutput=False)
    w_proj = nc.declare_dram_parameter("w_proj", [C, C], bf16, isOutput=False)
    b_proj = nc.declare_dram_parameter("b_proj", [C], f32, isOutput=False)
    tri = nc.declare_dram_parameter("tri", [P, P], bf16, isOutput=False)
    outT = nc.declare_dram_parameter("outT", [C, B * TSL], f32, isOutput=True)
    if debug_a2a:
        dbg = nc.declare_dram_parameter("dbg", [B * N_CORES * TSL, HL * HD],
                                        bf16, isOutput=True)

    xT_t = xT.rearrange("(kc p) t -> p kc t", p=P)          # [128,16,4096]
    wq_t = w_qkv.rearrange("(kc p) n -> p kc n", p=P)       # [128,16,768]
    wp_t = w_proj.rearrange("(kc p) n -> p kc n", p=P)      # [128,16,2048]

    scale = float(HD) ** -0.5

    with tile.TileContext(nc) as tc:
        with (
            # attention PSUM (banks 0-3): scores 2 + paired-o accum 2
            tc.tile_pool(name="attps", bufs=1, space="PSUM") as attps,
            tc.tile_pool(name="sb", bufs=1) as sb,
            tc.tile_pool(name="dram", bufs=1, space="DRAM") as dram,
        ):
            # ---- warmup-critical prefetch order: first x chunk, then the
            # first weight column, then everything else ----
            xc0 = sb.tile([P, KC, 512], bf16, tag="xc", bufs=2)
            for dq4 in range(4):
                nc.sync.dma_start(xc0[:, dq4 * 4:(dq4 + 1) * 4, :],
                                  xT_t[:, dq4 * 4:(dq4 + 1) * 4, 0:512])
            wq_sb = sb.tile([P, KC, 3 * HL * HD], bf16, tag="wq")
            for dq6 in range(6):
                nc.sync.dma_start(wq_sb[:, :, dq6 * P:(dq6 + 1) * P],
                                  wq_t[:, :, dq6 * P:(dq6 + 1) * P])
            bqk_sb = sb.tile([P, 4], f32, tag="bqk")         # q/k bias per col
            nc.sync.dma_start(bqk_sb[:],
                              b_qk.rearrange("(m p) -> p m", p=P))
            bp_sb = sb.tile([P, KC], f32, tag="bp")          # proj bias
            nc.sync.dma_start(bp_sb[:], b_proj.rearrange("(m p) -> p m", p=P))
            tri_sb = sb.tile([P, P], bf16, tag="tri")        # diag mask
            nc.sync.dma_start(tri_sb[:], tri[:, :])
            wp_sb = sb.tile([P, KC, C], bf16, tag="wp")      # resident w_proj

            a2a_in = [dram.tile([N_CORES, TSL, HL * HD], bf16, tag=f"ain{b}",
                                name=f"a2a_in{b}")
                      for b in range(B)]
            a2a_out = [dram.tile([N_CORES, TSL, HL * HD], bf16,
                                 tag=f"aout{b}", name=f"a2a_out{b}")
                       for b in range(B)]

            def emit_att(b, hl, qc, qk_sb, v_sb):
                qT_h = qk_sb[:, hl]              # [128, 2048]
                kT_h = qk_sb[:, 2 + hl]
                o4 = [attps.tile([P, HD + 1], f32, tag="o", bufs=4,
                                 name=f"o{jj}") for jj in range(4)]
                # units: (kb, wstart, first-diag?)
                units = [(kb, 0, False) for kb in range(4 * qc)]
                units += [(4 * qc + i, i * P, True) for i in range(4)]

                def emit_scores(u):
                    kb, ws, diag = u
                    s = attps.tile([P, 512], f32, tag="s", bufs=2)
                    nc.tensor.matmul(
                        s[:, ws:512],
                        kT_h[:, kb * P:(kb + 1) * P],
                        qT_h[:, qc * 512 + ws:(qc + 1) * 512],
                        start=True, stop=True)
                    probs = sb.tile([P, 512], bf16, tag="probs", bufs=3)
                    nc.scalar.activation(
                        out=probs[:, ws:512], in_=s[:, ws:512],
                        func=ACTF.Exp, scale=scale)
                    if diag:
                        nc.vector.tensor_tensor(
                            out=probs[:, ws:ws + P], in0=probs[:, ws:ws + P],
                            in1=tri_sb[:], op=ALU.mult)
                    return probs

                def emit_av(u, probs):
                    kb, ws, diag = u
                    j0 = ws // P if diag else 0
                    for j in range(j0, 4):
                        nc.tensor.matmul(
                            o4[j],
                            probs[:, j * P:(j + 1) * P],
                            v_sb[:, kb, hl, :],
                            start=(kb == 0),
                            stop=(kb == 4 * qc + j))

                prev = None
                for u in units:
                    probs = emit_scores(u)
                    if prev is not None:
                        emit_av(prev[0], prev[1])
                    prev = (u, probs)
                emit_av(prev[0], prev[1])

                ost = sb.tile([P, 4, HD], bf16, tag="ost", bufs=2)
                for j in range(4):
                    recip = sb.tile([P, 1], f32, tag="recip", bufs=4)
                    nc.vector.reciprocal(recip[:], o4[j][:, HD:HD + 1])
                    nc.vector.tensor_scalar_mul(
                        out=ost[:, j, :], in0=o4[j][:, 0:HD],
                        scalar1=recip[:])
                # one batched store: j -> chunk 2qc+j//2, row (j%2)*128
                dst = a2a_in[b].rearrange(
                    "(c2 c1) (t1 p) d -> p c2 c1 t1 d", c1=2, t1=2)
                nc.gpsimd.dma_start(
                    dst[:, qc, :, :, hl * HD:(hl + 1) * HD],
                    ost.rearrange("p (c1 t1) d -> p c1 t1 d", c1=2))

            for _rep in range(repeat):
              for b in range(B):
                qk_sb = sb.tile([P, 4, T], bf16, tag="qk", bufs=2)
                v_sb = sb.tile([P, KC, HL, HD + 1], bf16, tag="v", bufs=2)
                nc.vector.memset(v_sb[:, :, :, HD:HD + 1], 1.0)

                # ---- QKV projection interleaved with attention (qc = nb) ----
                with (
                    tc.tile_pool(name=f"qkv_ps{b}", bufs=2, space="PSUM") as qkvps,
                ):
                    for nb in range(NB):
                        g = b * T + nb * 512
                        if _rep == 0 and b == 0 and nb == 0:
                            xc = xc0        # prefetched before the weights
                        else:
                            xc = sb.tile([P, KC, 512], bf16, tag="xc", bufs=2)
                            for dq4 in range(4):
                                nc.sync.dma_start(
                                    xc[:, dq4 * 4:(dq4 + 1) * 4, :],
                                    xT_t[:, dq4 * 4:(dq4 + 1) * 4, g:g + 512])
                        for m in range(4):               # qh0 qh1 kh0 kh1
                            ps = qkvps.tile([P, 512], f32, tag="ps")
                            for kc in range(KC):
                                nc.tensor.matmul(
                                    ps[:],
                                    wq_sb[:, kc, m * P:(m + 1) * P],
                                    xc[:, kc, :],
                                    start=(kc == 0), stop=(kc == KC - 1))
                            nc.vector.tensor_scalar_add(
                                out=qk_sb[:, m, nb * 512:(nb + 1) * 512],
                                in0=ps[:], scalar1=bqk_sb[:, m:m + 1])
                        for tv in range(4):
                            ps = qkvps.tile([P, HL * HD], f32, tag="ps")
                            for kc in range(KC):
                                nc.tensor.matmul(
                                    ps[:],
                                    xc[:, kc, tv * P:(tv + 1) * P],
                                    wq_sb[:, kc, 2 * HL * HD:3 * HL * HD],
                                    start=(kc == 0), stop=(kc == KC - 1))
                            # [tok,256] -> v_sb [tok, head, 0:128]
                            nc.vector.tensor_copy(
                                v_sb[:, nb * 4 + tv, :, 0:HD], ps[:])
                        if _rep == 0 and b == 0 and nb == 1:
                            # w_proj load rides under early batch-0 compute
                            for dq4 in range(4):
                                nc.sync.dma_start(
                                    wp_sb[:, dq4 * 4:(dq4 + 1) * 4, :],
                                    wp_t[:, dq4 * 4:(dq4 + 1) * 4, :])
                        if with_att:
                            for hl in range(HL):
                                emit_att(b, hl, nb, qk_sb, v_sb)

                # ---- head exchange for this batch ----
                if debug_a2a:
                    nc.sync.dma_start(
                        dbg[b * N_CORES * TSL:(b + 1) * N_CORES * TSL, :],
                        a2a_in[b].rearrange("c t d -> (c t) d"))
                if with_collective:
                    nc.gpsimd.collective_compute(
                        "AllToAll", ALU.bypass,
                        replica_groups=[list(range(N_CORES))],
                        ins=[a2a_in[b].opt()], outs=[a2a_out[b].opt()])

              # ---- output projection on own token slices ----
              if not with_proj:
                  continue
              for b in range(B):
                  with tc.tile_pool(name=f"proj{b}", bufs=2,
                                    space="PSUM") as pps:
                      rhs_sb = sb.tile([P, KC, TSL], bf16, tag="rhs", bufs=2)
                      for p8 in range(N_CORES):
                          for h2 in range(HL):
                              nc.sync.dma_start(
                                  rhs_sb[:, HL * p8 + h2, :],
                                  a2a_out[b][p8, :, h2 * HD:(h2 + 1) * HD],
                                  transpose=True)
                      for m in range(KC):
                          ps = pps.tile([P, TSL], f32)
                          for kc in range(KC):
                              nc.tensor.matmul(ps[:], wp_sb[:, kc,
                                                            m * P:(m + 1) * P],
                                               rhs_sb[:, kc, :],
                                               start=(kc == 0),
                                               stop=(kc == KC - 1))
                          o = sb.tile([P, TSL], f32, tag="pout", bufs=2)
                          nc.scalar.activation(out=o[:], in_=ps[:],
                                               func=ACTF.Identity,
                                               bias=bp_sb[:, m:m + 1],
                                               scale=1.0)
                          nc.sync.dma_start(
                              outT[m * P:(m + 1) * P, b * TSL:(b + 1) * TSL],
                              o[:])

    _split_multi_waits(nc)
    return nc


def _host_inputs(x, w_attn, b_attn, w_proj, b_proj):
    x = np.asarray(x, dtype=np.float32)
    w_attn = np.asarray(w_attn, dtype=np.float32)
    b_attn = np.asarray(b_attn, dtype=np.float32)
    w_proj = np.asarray(w_proj, dtype=np.float32)
    b_proj = np.ascontiguousarray(np.asarray(b_proj, dtype=np.float32))

    xT = np.ascontiguousarray(x.reshape(BT, C).T.astype(bfnp))
    w_proj_bf = np.ascontiguousarray(w_proj.astype(bfnp))
    # v-bias folds through the projection: out = (P V)/r @ W + b_v @ W + b_p
    b_proj = b_proj + b_attn[2 * C:3 * C] @ w_proj
    b_proj = np.ascontiguousarray(b_proj)

    # tri[r, c] = 1 where query col >= key row (within a diagonal block)
    cols = np.arange(P)
    tri = (cols[None, :] >= cols[:, None]).astype(bfnp)

    in_maps = []
    for c in range(N_CORES):
        col = HL * c * HD
        w_qkv = np.concatenate(
            [w_attn[:, col:col + HL * HD],
             w_attn[:, C + col:C + col + HL * HD],
             w_attn[:, 2 * C + col:2 * C + col + HL * HD]], axis=1)
        b_qk = np.concatenate(
            [b_attn[col:col + HL * HD],
             b_attn[C + col:C + col + HL * HD]])
        in_maps.append({
            "xT": xT,
            "w_qkv": np.ascontiguousarray(w_qkv.astype(bfnp)),
            "b_qk": np.ascontiguousarray(b_qk),
            "w_proj": w_proj_bf,
            "b_proj": b_proj,
            "tri": tri,
        })
    return in_maps


def kernel(x, w_attn, b_attn, w_proj, b_proj, _results_out=None):
    if "nc" not in _CACHE:
        _CACHE["nc"] = _build_bass()
    nc = _CACHE["nc"]
    in_maps = _host_inputs(x, w_attn, b_attn, w_proj, b_proj)
    res = run_bass_kernel_spmd(nc, in_maps, list(range(N_CORES)))
    if _results_out is not None:
        _results_out.append(res)
    out = np.empty((B, T, C), dtype=np.float32)
    for c in range(N_CORES):
        oc = res.results[c]["outT"]                      # [C, 2*256]
        for b in range(B):
            out[b, c * TSL:(c + 1) * TSL, :] = oc[:, b * TSL:(b + 1) * TSL].T
    return out
